# revision 7
# baseline (speedup 1.0000x reference)
"""Trainium2 Bass kernel for nn_EnhancedBVHRouter (hierarchical MoE router).

Strategy:
  - Pure data parallel over batch: 32768 rows -> 8 cores x 4096 rows.
  - All activations kept TRANSPOSED on-chip ([features on partitions, rows on
    free dim]) so every linear layer is `out_T = W^T @ in_T` with the weight
    matrix (already [din, dout] in HBM) as the stationary matmul operand and
    no on-chip transposes of activations anywhere in the main chain.
  - Matmul operands are bf16 (full PE rate); all accumulation is f32 in PSUM.
    Measured offline: bf16 operand rounding gives max |probs err| ~4e-5 and
    zero argmax flips (top-2 logit gap is >= 3.2e-4 across the whole batch).
  - The per-level geometric term is folded into the router weights on the
    host:  0.5*geo_j = -gs*|pos|^2 + 2*gs*pos.c_j - gs*|c_j|^2  with
    gs = 0.5/(2*T^2+1e-8).  The -gs*|pos|^2 term is constant per row and
    softmax-invariant, so it is dropped; the rest becomes rW'/rb'.
  - LayerNorm / level-softmax partition reductions are done with ones-vector
    matmuls on the PE; per-row scalars are broadcast back across partitions
    with gpsimd.partition_broadcast.
  - Final 64-way softmax + argmax run row-major after a PE transpose of the
    [64, R] logits; probs stream straight out to HBM, argmax indices are
    collected as f32, transposed once per core, and written as int32.
"""

import os
import sys

import numpy as np

for _p in ("/opt/trn_rl_repo", "/root/.axon_site/_ro/trn_rl_repo"):
    if os.path.isdir(_p) and _p not in sys.path:
        sys.path.append(_p)

import ml_dtypes

BF16 = ml_dtypes.bfloat16

B, D_IN, FD = 32768, 2048, 128
N_CORES = 8
BLOC = B // N_CORES          # 4096 rows per core
R = 512                      # rows per macro tile
NRT = BLOC // R              # 8 macro tiles per core
KO = D_IN // 128             # 16 k-chunks for the input projection
NT = BLOC // 128             # 32 argmax columns per core
T_BUF = 1.0
LN_EPS = 1e-5
GS = 0.5 / (2.0 * T_BUF * T_BUF + 1e-8)
N_EXP = 64

_PROGRAM = None


def _build_program():
    import concourse.tile as tile
    import concourse.mybir as mybir
    from concourse import bacc
    from concourse.masks import make_identity
    from contextlib import ExitStack

    f32 = mybir.dt.float32
    bf16 = mybir.dt.bfloat16
    AF = mybir.ActivationFunctionType
    OP = mybir.AluOpType

    nc = bacc.Bacc("TRN2", target_bir_lowering=False, debug=False)

    # ---- DRAM tensors (names are the in_map keys) ----
    xT_d = nc.dram_tensor("xT", [KO, 128, BLOC], bf16, kind="ExternalInput")
    w1_d = nc.dram_tensor("w1", [128, KO, 4, 128], bf16, kind="ExternalInput")
    b1_d = nc.dram_tensor("b1", [128, 4], f32, kind="ExternalInput")
    w2_d = nc.dram_tensor("w2", [128, 4, 2, 128], bf16, kind="ExternalInput")
    b2_d = nc.dram_tensor("b2", [128, 2], f32, kind="ExternalInput")
    lng_d = nc.dram_tensor("lng", [128, 2], f32, kind="ExternalInput")
    lnb_d = nc.dram_tensor("lnb", [128, 2], f32, kind="ExternalInput")
    lv_d = []
    for i, dk in ((1, 2), (2, 1), (3, 1)):
        lv_d.append({
            "w3d": nc.dram_tensor(f"l{i}w3d", [128, dk, 3], bf16, kind="ExternalInput"),
            "b3d": nc.dram_tensor(f"l{i}b3d", [3, 1], f32, kind="ExternalInput"),
            "f1w": nc.dram_tensor(f"l{i}f1w", [128, dk, 128], bf16, kind="ExternalInput"),
            "f1b": nc.dram_tensor(f"l{i}f1b", [128, 1], f32, kind="ExternalInput"),
            "f2w": nc.dram_tensor(f"l{i}f2w", [128, 128], bf16, kind="ExternalInput"),
            "f2b": nc.dram_tensor(f"l{i}f2b", [128, 1], f32, kind="ExternalInput"),
            "rwf": nc.dram_tensor(f"l{i}rwf", [128, 4], bf16, kind="ExternalInput"),
            "rwp": nc.dram_tensor(f"l{i}rwp", [3, 4], bf16, kind="ExternalInput"),
            "rb": nc.dram_tensor(f"l{i}rb", [4, 1], f32, kind="ExternalInput"),
            "dk": dk,
        })
    ehw1f_d = nc.dram_tensor("ehw1f", [128, 2, 128], bf16, kind="ExternalInput")
    ehw1p_d = [nc.dram_tensor(f"ehw1p{l}", [4, 2, 128], bf16, kind="ExternalInput")
               for l in range(3)]
    ehb1_d = nc.dram_tensor("ehb1", [128, 2], f32, kind="ExternalInput")
    ehw2_d = nc.dram_tensor("ehw2", [128, 2, N_EXP], bf16, kind="ExternalInput")
    ehb2_d = nc.dram_tensor("ehb2", [N_EXP, 1], f32, kind="ExternalInput")

    probs_d = nc.dram_tensor("probs", [BLOC, N_EXP], f32, kind="ExternalOutput")
    ids_d = nc.dram_tensor("ids", [NT, 128], mybir.dt.int32, kind="ExternalOutput")

    with tile.TileContext(nc) as tc, ExitStack() as ctx:
        singles = ctx.enter_context(tc.tile_pool(name="singles", bufs=1))
        sb = ctx.enter_context(tc.tile_pool(name="sb", bufs=2))
        sbs = ctx.enter_context(tc.tile_pool(name="sbs", bufs=3))
        pbig = ctx.enter_context(tc.tile_pool(name="pbig", bufs=4, space="PSUM"))
        psm = ctx.enter_context(tc.tile_pool(name="psm", bufs=2, space="PSUM"))
        pmx = ctx.enter_context(tc.tile_pool(name="pmx", bufs=2, space="PSUM"))

        def load(dram, shape, dtype):
            t = singles.tile(shape, dtype, tag=f"w_{dram.name}")
            nc.sync.dma_start(out=t, in_=dram.ap())
            return t

        w1 = load(w1_d, [128, KO, 4, 128], bf16)
        b1 = load(b1_d, [128, 4], f32)
        w2 = load(w2_d, [128, 4, 2, 128], bf16)
        b2 = load(b2_d, [128, 2], f32)
        lng = load(lng_d, [128, 2], f32)
        lnb = load(lnb_d, [128, 2], f32)
        lv = []
        for i, d in enumerate(lv_d):
            dk = d["dk"]
            lv.append({
                "w3d": load(d["w3d"], [128, dk, 3], bf16),
                "b3d": load(d["b3d"], [3, 1], f32),
                "f1w": load(d["f1w"], [128, dk, 128], bf16),
                "f1b": load(d["f1b"], [128, 1], f32),
                "f2w": load(d["f2w"], [128, 128], bf16),
                "f2b": load(d["f2b"], [128, 1], f32),
                "rwf": load(d["rwf"], [128, 4], bf16),
                "rwp": load(d["rwp"], [3, 4], bf16),
                "rb": load(d["rb"], [4, 1], f32),
                "dk": dk,
            })
        ehw1f = load(ehw1f_d, [128, 2, 128], bf16)
        ehw1p = [load(d, [4, 2, 128], bf16) for d in ehw1p_d]
        ehb1 = load(ehb1_d, [128, 2], f32)
        ehw2 = load(ehw2_d, [128, 2, N_EXP], bf16)
        ehb2 = load(ehb2_d, [N_EXP, 1], f32)

        ones = singles.tile([128, 1], bf16)
        nc.vector.memset(ones, 1.0)
        epsb = singles.tile([1, 1], f32)
        nc.vector.memset(epsb, LN_EPS)
        ident = singles.tile([128, 128], f32)
        make_identity(nc, ident)
        idsf = singles.tile([128, NT], f32)

        xT_ap = xT_d.ap().rearrange("k p b -> p k b")

        def level(i, chunks):
            """chunks: list of [128, R] bf16 APs (the transposed input).
            Returns (feat [128, R] bf16, p [4, R] bf16)."""
            d = lv[i]
            dk = d["dk"]
            pos_ps = psm.tile([4, R], f32, tag="small")
            for ko in range(dk):
                nc.tensor.matmul(pos_ps[0:3, :], d["w3d"][:, ko, :], chunks[ko],
                                 start=(ko == 0), stop=(ko == dk - 1))
            pos_sb = sbs.tile([3, R], bf16, tag="pos")
            nc.scalar.activation(pos_sb, pos_ps[0:3, :], AF.Identity, bias=d["b3d"])
            pf = pbig.tile([128, R], f32, tag="big")
            for ko in range(dk):
                nc.tensor.matmul(pf, d["f1w"][:, ko, :], chunks[ko],
                                 start=(ko == 0), stop=(ko == dk - 1))
            g = sbs.tile([128, R], bf16, tag="g")
            nc.scalar.activation(g, pf, AF.Gelu, bias=d["f1b"])
            pfeat = pbig.tile([128, R], f32, tag="big")
            nc.tensor.matmul(pfeat, d["f2w"], g, start=True, stop=True)
            feat = sb.tile([128, R], bf16, tag=f"feat{i}")
            nc.scalar.activation(feat, pfeat, AF.Identity, bias=d["f2b"])
            plog = psm.tile([4, R], f32, tag="small")
            nc.tensor.matmul(plog, d["rwf"], feat, start=True, stop=False)
            nc.tensor.matmul(plog, d["rwp"], pos_sb, start=False, stop=True)
            e32 = sbs.tile([4, R], f32, tag="e32")
            nc.scalar.activation(e32, plog, AF.Exp, bias=d["rb"])
            e16 = sbs.tile([4, R], bf16, tag="e16")
            nc.vector.tensor_copy(out=e16, in_=e32)
            pse = psm.tile([1, R], f32, tag="small")
            nc.tensor.matmul(pse, ones[0:4, :], e16, start=True, stop=True)
            rec = sbs.tile([1, R], f32, tag="rec")
            nc.vector.reciprocal(rec, pse)
            recb = sbs.tile([4, R], f32, tag="recb")
            nc.gpsimd.partition_broadcast(recb, rec, channels=4)
            p_l = sb.tile([4, R], bf16, tag=f"p{i}")
            nc.vector.tensor_tensor(p_l, e32, recb, OP.mult)
            return feat, p_l

        for t in range(NRT):
            r0 = t * R
            xt = sb.tile([128, KO, R], bf16, tag="xt")
            nc.sync.dma_start(out=xt, in_=xT_ap[:, :, r0:r0 + R])

            # input_proj layer 1: h1T = gelu(W1^T x + b1)   [512, R]
            g1 = sb.tile([128, 4, R], bf16, tag="g1")
            for mo in range(4):
                ph = pbig.tile([128, R], f32, tag="big")
                for ko in range(KO):
                    nc.tensor.matmul(ph, w1[:, ko, mo, :], xt[:, ko, :],
                                     start=(ko == 0), stop=(ko == KO - 1))
                nc.scalar.activation(g1[:, mo, :], ph, AF.Gelu, bias=b1[:, mo:mo + 1])

            # input_proj layer 2 + LayerNorm stats
            h2 = sb.tile([128, 2, R], bf16, tag="h2")
            h2q = sb.tile([128, 2, R], bf16, tag="h2q")
            for mo in range(2):
                ph = pbig.tile([128, R], f32, tag="big")
                for ko in range(4):
                    nc.tensor.matmul(ph, w2[:, ko, mo, :], g1[:, ko, :],
                                     start=(ko == 0), stop=(ko == 3))
                nc.scalar.activation(h2[:, mo, :], ph, AF.Identity, bias=b2[:, mo:mo + 1])
                nc.scalar.activation(h2q[:, mo, :], ph, AF.Square, bias=b2[:, mo:mo + 1])
            ps_s = psm.tile([1, R], f32, tag="small")
            for mo in range(2):
                nc.tensor.matmul(ps_s, ones, h2[:, mo, :], start=(mo == 0), stop=(mo == 1))
            ps_s2 = psm.tile([1, R], f32, tag="small")
            for mo in range(2):
                nc.tensor.matmul(ps_s2, ones, h2q[:, mo, :], start=(mo == 0), stop=(mo == 1))
            mu = sbs.tile([1, R], f32, tag="mu")
            nc.scalar.mul(mu, ps_s, 1.0 / 256.0)
            e2m = sbs.tile([1, R], f32, tag="e2m")
            nc.scalar.mul(e2m, ps_s2, 1.0 / 256.0)
            var = sbs.tile([1, R], f32, tag="var")
            nc.vector.tensor_tensor(var, mu, mu, OP.mult)
            nc.vector.tensor_tensor(var, e2m, var, OP.subtract)
            sd = sbs.tile([1, R], f32, tag="sd")
            nc.scalar.activation(sd, var, AF.Sqrt, bias=epsb)
            rstd = sbs.tile([1, R], f32, tag="rstd")
            nc.vector.reciprocal(rstd, sd)
            mub = sb.tile([128, R], f32, tag="mub")
            nc.gpsimd.partition_broadcast(mub, mu, channels=128)
            rstdb = sb.tile([128, R], f32, tag="rstdb")
            nc.gpsimd.partition_broadcast(rstdb, rstd, channels=128)
            hn = sb.tile([128, 2, R], bf16, tag="hn")
            for mo in range(2):
                zc = sb.tile([128, R], f32, tag="zc")
                nc.vector.tensor_tensor(zc, h2[:, mo, :], mub, OP.subtract)
                nc.vector.tensor_tensor(zc, zc, rstdb, OP.mult)
                nc.vector.tensor_scalar(out=hn[:, mo, :], in0=zc,
                                        scalar1=lng[:, mo:mo + 1], scalar2=lnb[:, mo:mo + 1],
                                        op0=OP.mult, op1=OP.add)

            # levels
            f1, p1 = level(0, [hn[:, 0, :], hn[:, 1, :]])
            f2, p2 = level(1, [f1])
            f3, p3 = level(2, [f2])
            ps = (p1, p2, p3)

            # expert head
            ge = sb.tile([128, 2, R], bf16, tag="ge")
            for mo in range(2):
                pe1 = pbig.tile([128, R], f32, tag="big")
                nc.tensor.matmul(pe1, ehw1f[:, mo, :], f3, start=True, stop=False)
                for l in range(3):
                    nc.tensor.matmul(pe1, ehw1p[l][:, mo, :], ps[l],
                                     start=False, stop=(l == 2))
                nc.scalar.activation(ge[:, mo, :], pe1, AF.Gelu, bias=ehb1[:, mo:mo + 1])
            pl2 = pmx.tile([N_EXP, R], f32, tag="pmix")
            for mo in range(2):
                nc.tensor.matmul(pl2, ehw2[:, mo, :], ge[:, mo, :],
                                 start=(mo == 0), stop=(mo == 1))
            lsb = sb.tile([N_EXP, R], f32, tag="lsb")
            nc.vector.tensor_copy(out=lsb, in_=pl2)

            # final softmax + argmax, row-major per 128-row subtile
            for sub in range(4):
                pt = pmx.tile([128, N_EXP], f32, tag="pmix")
                nc.tensor.transpose(pt, lsb[:, sub * 128:(sub + 1) * 128],
                                    ident[0:N_EXP, 0:N_EXP])
                ls = sbs.tile([128, N_EXP], f32, tag="ls")
                nc.vector.tensor_copy(out=ls, in_=pt)
                m8 = sbs.tile([128, 8], f32, tag="m8")
                nc.vector.max(m8, ls)
                i8 = sbs.tile([128, 8], mybir.dt.uint32, tag="i8")
                nc.vector.max_index(i8, m8, ls)
                negm = sbs.tile([128, 1], f32, tag="negm")
                nc.vector.tensor_scalar_mul(negm, m8[:, 0:1], -1.0)
                es = sbs.tile([128, N_EXP], f32, tag="es")
                ssum = sbs.tile([128, 1], f32, tag="ssum")
                nc.scalar.activation(es, ls, AF.Exp, bias=negm, accum_out=ssum)
                srec = sbs.tile([128, 1], f32, tag="srec")
                nc.vector.reciprocal(srec, ssum)
                prb = sbs.tile([128, N_EXP], f32, tag="prb")
                nc.vector.tensor_scalar_mul(prb, es, srec)
                nc.sync.dma_start(out=probs_d.ap()[r0 + sub * 128: r0 + (sub + 1) * 128, :],
                                  in_=prb)
                c = t * 4 + sub
                nc.vector.tensor_copy(out=idsf[:, c:c + 1], in_=i8[:, 0:1])

        pti = pmx.tile([NT, 128], f32, tag="pmix")
        nc.tensor.transpose(pti, idsf, ident)
        idsi = sbs.tile([NT, 128], mybir.dt.int32, tag="idsi")
        nc.vector.tensor_copy(out=idsi, in_=pti)
        nc.sync.dma_start(out=ids_d.ap(), in_=idsi)

    nc.finalize()
    return nc


def get_program():
    global _PROGRAM
    if _PROGRAM is None:
        _PROGRAM = _build_program()
    return _PROGRAM


def prep_inputs(inputs):
    """Host-side: fold geo term, cast to bf16, lay out weights for the SBUF
    layouts above; shard x. Returns list of 8 per-core in_maps."""
    f32 = np.float32
    inp = {k: np.asarray(v) for k, v in inputs.items()}

    def c_(a):
        return np.ascontiguousarray(a)

    shared = {}
    shared["w1"] = inp["ipW1"].reshape(KO, 128, 4, 128).transpose(1, 0, 2, 3).astype(BF16)
    shared["b1"] = c_(inp["ipb1"].astype(f32).reshape(4, 128).T)
    shared["w2"] = inp["ipW2"].reshape(4, 128, 2, 128).transpose(1, 0, 2, 3).astype(BF16)
    shared["b2"] = c_(inp["ipb2"].astype(f32).reshape(2, 128).T)
    shared["lng"] = c_(inp["ln_g"].astype(f32).reshape(2, 128).T)
    shared["lnb"] = c_(inp["ln_b"].astype(f32).reshape(2, 128).T)
    for i, dk in ((1, 2), (2, 1), (3, 1)):
        p = f"l{i}"
        shared[f"{p}w3d"] = inp[p + "W3d"].reshape(dk, 128, 3).transpose(1, 0, 2).astype(BF16)
        shared[f"{p}b3d"] = c_(inp[p + "b3d"].astype(f32).reshape(3, 1))
        shared[f"{p}f1w"] = inp[p + "f1W"].reshape(dk, 128, 128).transpose(1, 0, 2).astype(BF16)
        shared[f"{p}f1b"] = c_(inp[p + "f1b"].astype(f32).reshape(128, 1))
        shared[f"{p}f2w"] = inp[p + "f2W"].astype(BF16)
        shared[f"{p}f2b"] = c_(inp[p + "f2b"].astype(f32).reshape(128, 1))
        cc = inp[p + "c"].astype(np.float64)             # [4, 3]
        rW = inp[p + "rW"].astype(np.float64)            # [131, 4]
        shared[f"{p}rwf"] = inp[p + "rW"][:FD].astype(BF16)
        shared[f"{p}rwp"] = (rW[FD:FD + 3] + 2.0 * GS * cc.T).astype(f32).astype(BF16)
        rb2 = inp[p + "rb"].astype(np.float64) - GS * (cc ** 2).sum(1)
        shared[f"{p}rb"] = c_(rb2.astype(f32).reshape(4, 1))
    shared["ehw1f"] = inp["ehW1"][:FD].reshape(128, 2, 128).astype(BF16)
    for l in range(3):
        shared[f"ehw1p{l}"] = inp["ehW1"][FD + 4 * l:FD + 4 * l + 4].reshape(4, 2, 128).astype(BF16)
    shared["ehb1"] = c_(inp["ehb1"].astype(f32).reshape(2, 128).T)
    shared["ehw2"] = inp["ehW2"].reshape(2, 128, N_EXP).transpose(1, 0, 2).astype(BF16)
    shared["ehb2"] = c_(inp["ehb2"].astype(f32).reshape(N_EXP, 1))

    xb = inp["x"].astype(BF16)
    maps = []
    for core in range(N_CORES):
        xc = xb[core * BLOC:(core + 1) * BLOC]           # [BLOC, D_IN]
        xT = c_(xc.T).reshape(KO, 128, BLOC)             # [ko, p, b]
        m = dict(shared)
        m["xT"] = xT
        maps.append(m)
    return maps


def run(inputs, trace=False, **kw):
    from concourse.bass_utils import run_bass_kernel_spmd
    nc = get_program()
    in_maps = prep_inputs(inputs)
    res = run_bass_kernel_spmd(nc, in_maps, core_ids=list(range(N_CORES)),
                               trace=trace, **kw)
    probs = np.concatenate([r["probs"] for r in res.results], axis=0)
    ids = np.concatenate([np.ascontiguousarray(r["ids"]).reshape(-1)
                          for r in res.results], axis=0).astype(np.int32)
    return (probs, ids), res


def kernel(**inputs):
    (probs, ids), _ = run(inputs, trace=False)
    return probs, ids


# revision 9
# speedup vs baseline: 1.3927x; 1.3927x over previous
"""Trainium2 Bass kernel for nn_EnhancedBVHRouter (hierarchical MoE router).

Strategy:
  - Pure data parallel over batch: 32768 rows -> 8 cores x 4096 rows.
  - All activations kept TRANSPOSED on-chip ([features on partitions, rows on
    free dim]) so every linear layer is `out_T = W^T @ in_T` with the weight
    matrix as the stationary matmul operand; no activation transposes in the
    main chain.  Matmul operands are bf16 (full PE rate), f32 PSUM accumulate.
  - The level geometric term and every intermediate bias are folded on the
    host:  0.5*geo_j = -gs*|pos|^2 + 2*gs*pos.c_j - gs*|c_j|^2 drops its
    row-constant |pos|^2 term (softmax invariant); the pos path itself is
    folded into one combined matrix Wcomb = W3d @ rW_pos', so `pos` is never
    materialized on chip.
  - LayerNorm / level-softmax partition reductions are ones-matmuls on the PE
    (ones scaled 1/256 so the PSUM result IS the mean); per-row scalars come
    back via gpsimd.partition_broadcast; reciprocals use the single-op
    reciprocal_approx_fast (~5x faster than InstReciprocal).
  - Two row-tiles are processed per "group" phase-by-phase so same-function
    ACT ops batch together (the Gelu/Exp/Sqrt LUT table reload costs ~1.3us
    per switch).
  - Final 64-way softmax (no max subtraction needed: |logits| < 0.01) +
    argmax run row-major after a PE transpose of the [64, R] logits.
"""

import os
import sys

import numpy as np

for _p in ("/opt/trn_rl_repo", "/root/.axon_site/_ro/trn_rl_repo"):
    if os.path.isdir(_p) and _p not in sys.path:
        sys.path.append(_p)

import ml_dtypes

BF16 = ml_dtypes.bfloat16

B, D_IN, FD = 32768, 2048, 128
N_CORES = 8
BLOC = B // N_CORES          # 4096 rows per core
R = 512                      # rows per macro tile
NRT = BLOC // R              # 8 macro tiles per core
KO = D_IN // 128             # 16 k-chunks for the input projection
NT = BLOC // 128             # 32 argmax columns per core
GRP = 2                      # row-tiles per phase group
T_BUF = 1.0
LN_EPS = 1e-5
GS = 0.5 / (2.0 * T_BUF * T_BUF + 1e-8)
N_EXP = 64
LV_DK = (2, 1, 1)            # k-chunks of each level's input

_PROGRAM = None


def _build_program():
    import concourse.tile as tile
    import concourse.mybir as mybir
    from concourse import bacc
    from concourse.masks import make_identity
    from contextlib import ExitStack

    f32 = mybir.dt.float32
    bf16 = mybir.dt.bfloat16
    AF = mybir.ActivationFunctionType
    OP = mybir.AluOpType

    nc = bacc.Bacc("TRN2", target_bir_lowering=False, debug=False)

    # ---- DRAM tensors (names are the in_map keys) ----
    xT_d = nc.dram_tensor("xT", [NRT, 128, KO, R], bf16, kind="ExternalInput")
    w1_d = nc.dram_tensor("w1", [128, KO, 4, 128], bf16, kind="ExternalInput")
    b1_d = nc.dram_tensor("b1", [128, 4], f32, kind="ExternalInput")
    w2_d = nc.dram_tensor("w2", [128, 4, 2, 128], bf16, kind="ExternalInput")
    b2_d = nc.dram_tensor("b2", [128, 2], f32, kind="ExternalInput")
    lng_d = nc.dram_tensor("lng", [128, 2], f32, kind="ExternalInput")
    lnb_d = nc.dram_tensor("lnb", [128, 2], f32, kind="ExternalInput")
    lv_d = []
    for i in range(3):
        dk = LV_DK[i]
        lv_d.append({
            "f1w": nc.dram_tensor(f"l{i}f1w", [128, dk, 128], bf16, kind="ExternalInput"),
            "f1b": nc.dram_tensor(f"l{i}f1b", [128, 1], f32, kind="ExternalInput"),
            "f2w": nc.dram_tensor(f"l{i}f2w", [128, 128], bf16, kind="ExternalInput"),
            "wcomb": nc.dram_tensor(f"l{i}wcomb", [128, dk, 4], bf16, kind="ExternalInput"),
            "rwf": nc.dram_tensor(f"l{i}rwf", [128, 4], bf16, kind="ExternalInput"),
            "rb": nc.dram_tensor(f"l{i}rb", [4, 1], f32, kind="ExternalInput"),
        })
    ehw1f_d = nc.dram_tensor("ehw1f", [128, 2, 128], bf16, kind="ExternalInput")
    ehw1p_d = [nc.dram_tensor(f"ehw1p{l}", [4, 2, 128], bf16, kind="ExternalInput")
               for l in range(3)]
    ehb1_d = nc.dram_tensor("ehb1", [128, 2], f32, kind="ExternalInput")
    ehw2_d = nc.dram_tensor("ehw2", [128, 2, N_EXP], bf16, kind="ExternalInput")
    ehb2_d = nc.dram_tensor("ehb2", [N_EXP, 1], f32, kind="ExternalInput")

    probs_d = nc.dram_tensor("probs", [BLOC, N_EXP], f32, kind="ExternalOutput")
    ids_d = nc.dram_tensor("ids", [NT, 128], mybir.dt.int32, kind="ExternalOutput")

    with tile.TileContext(nc) as tc, ExitStack() as ctx:
        singles = ctx.enter_context(tc.tile_pool(name="singles", bufs=1))
        sb = ctx.enter_context(tc.tile_pool(name="sb", bufs=2))
        sbs = ctx.enter_context(tc.tile_pool(name="sbs", bufs=3))
        pbig = ctx.enter_context(tc.tile_pool(name="pbig", bufs=4, space="PSUM"))
        psm = ctx.enter_context(tc.tile_pool(name="psm", bufs=2, space="PSUM"))
        pmx = ctx.enter_context(tc.tile_pool(name="pmx", bufs=2, space="PSUM"))

        def load(dram, shape, dtype):
            t = singles.tile(shape, dtype, tag=f"w_{dram.name}")
            nc.sync.dma_start(out=t, in_=dram.ap())
            return t

        w1 = load(w1_d, [128, KO, 4, 128], bf16)
        b1 = load(b1_d, [128, 4], f32)
        w2 = load(w2_d, [128, 4, 2, 128], bf16)
        b2 = load(b2_d, [128, 2], f32)
        lng = load(lng_d, [128, 2], f32)
        lnb = load(lnb_d, [128, 2], f32)
        lv = []
        for i, d in enumerate(lv_d):
            dk = LV_DK[i]
            lv.append({
                "f1w": load(d["f1w"], [128, dk, 128], bf16),
                "f1b": load(d["f1b"], [128, 1], f32),
                "f2w": load(d["f2w"], [128, 128], bf16),
                "wcomb": load(d["wcomb"], [128, dk, 4], bf16),
                "rwf": load(d["rwf"], [128, 4], bf16),
                "rb": load(d["rb"], [4, 1], f32),
            })
        ehw1f = load(ehw1f_d, [128, 2, 128], bf16)
        ehw1p = [load(d, [4, 2, 128], bf16) for d in ehw1p_d]
        ehb1 = load(ehb1_d, [128, 2], f32)
        ehw2 = load(ehw2_d, [128, 2, N_EXP], bf16)
        ehb2 = load(ehb2_d, [N_EXP, 1], f32)

        # ones/256 so the stats matmuls produce means directly
        onesm = singles.tile([128, 1], bf16)
        nc.vector.memset(onesm, 1.0 / 256.0)
        ones4 = singles.tile([4, 1], bf16)
        nc.vector.memset(ones4, 1.0)
        epsb = singles.tile([1, 1], f32)
        nc.vector.memset(epsb, LN_EPS)
        ident = singles.tile([128, 128], f32)
        make_identity(nc, ident)
        idsf = singles.tile([128, NT], f32)

        for tp in range(0, NRT, GRP):
            grp = list(range(tp, min(tp + GRP, NRT)))

            # ---------- phase 1: input_proj layer 1 (64 MMs + gelu per tile)
            xt = {}
            g1 = {}
            for t in grp:
                xt[t] = sb.tile([128, KO, R], bf16, tag="xt", bufs=3, name=f"xt{t}")
                nc.sync.dma_start(out=xt[t], in_=xT_d.ap()[t])
            for t in grp:
                g1[t] = sb.tile([128, 4, R], bf16, tag="g1", bufs=2, name=f"g1_{t}")
                for mo in range(4):
                    ph = pbig.tile([128, R], f32, tag="big", name=f"ph1_{t}_{mo}")
                    for ko in range(KO):
                        nc.tensor.matmul(ph, w1[:, ko, mo, :], xt[t][:, ko, :],
                                         start=(ko == 0), stop=(ko == KO - 1))
                    nc.scalar.activation(g1[t][:, mo, :], ph, AF.Gelu, bias=b1[:, mo:mo + 1])

            # ---------- phase 2: input_proj layer 2 + LayerNorm
            hn = {}
            for t in grp:
                h2 = sb.tile([128, 2, R], bf16, tag="h2", name=f"h2_{t}")
                h2q = sb.tile([128, 2, R], bf16, tag="h2q", name=f"h2q_{t}")
                for mo in range(2):
                    ph = pbig.tile([128, R], f32, tag="big", name=f"ph2_{t}_{mo}")
                    for ko in range(4):
                        nc.tensor.matmul(ph, w2[:, ko, mo, :], g1[t][:, ko, :],
                                         start=(ko == 0), stop=(ko == 3))
                    nc.vector.tensor_scalar_add(h2[:, mo, :], ph, b2[:, mo:mo + 1])
                    nc.vector.tensor_tensor(h2q[:, mo, :], h2[:, mo, :], h2[:, mo, :], OP.mult)
                mu_ps = psm.tile([1, R], f32, tag="small", name=f"mu_ps_{t}")
                for mo in range(2):
                    nc.tensor.matmul(mu_ps, onesm, h2[:, mo, :], start=(mo == 0), stop=(mo == 1))
                e2_ps = psm.tile([1, R], f32, tag="small", name=f"e2_ps_{t}")
                for mo in range(2):
                    nc.tensor.matmul(e2_ps, onesm, h2q[:, mo, :], start=(mo == 0), stop=(mo == 1))
                musb = sbs.tile([1, R], f32, tag="musb", name=f"musb_{t}")
                nc.scalar.copy(musb, mu_ps)
                var = sbs.tile([1, R], f32, tag="var", name=f"var_{t}")
                nc.vector.tensor_tensor(var, musb, musb, OP.mult)
                nc.vector.tensor_tensor(var, e2_ps, var, OP.subtract)
                sd = sbs.tile([1, R], f32, tag="sd", name=f"sd_{t}")
                nc.scalar.activation(sd, var, AF.Sqrt, bias=epsb)
                rstd = sbs.tile([1, R], f32, tag="rstd", name=f"rstd_{t}")
                nc.vector.reciprocal_approx_fast(out=rstd, in_=sd)
                mub = sb.tile([128, R], f32, tag="mub", name=f"mub_{t}")
                nc.gpsimd.partition_broadcast(mub, musb, channels=128)
                rstdb = sb.tile([128, R], f32, tag="rstdb", name=f"rstdb_{t}")
                nc.gpsimd.partition_broadcast(rstdb, rstd, channels=128)
                hn[t] = sb.tile([128, 2, R], bf16, tag="hn", name=f"hn_{t}")
                for mo in range(2):
                    zc = sb.tile([128, R], f32, tag="zc", name=f"zc_{t}_{mo}")
                    nc.vector.tensor_tensor(zc, h2[:, mo, :], mub, OP.subtract)
                    nc.vector.tensor_tensor(zc, zc, rstdb, OP.mult)
                    nc.vector.tensor_scalar(out=hn[t][:, mo, :], in0=zc,
                                            scalar1=lng[:, mo:mo + 1], scalar2=lnb[:, mo:mo + 1],
                                            op0=OP.mult, op1=OP.add)

            # ---------- phase 3: level feature chains (all gelus together)
            feats = {}
            for t in grp:
                inp = [hn[t][:, 0, :], hn[t][:, 1, :]]
                fl = []
                for i in range(3):
                    d = lv[i]
                    dk = LV_DK[i]
                    pf = pbig.tile([128, R], f32, tag="big", name=f"pf_{t}_{i}")
                    for ko in range(dk):
                        nc.tensor.matmul(pf, d["f1w"][:, ko, :], inp[ko],
                                         start=(ko == 0), stop=(ko == dk - 1))
                    g = sbs.tile([128, R], bf16, tag="g", name=f"g_{t}_{i}")
                    nc.scalar.activation(g, pf, AF.Gelu, bias=d["f1b"])
                    pfeat = pbig.tile([128, R], f32, tag="big", name=f"pfeat_{t}_{i}")
                    nc.tensor.matmul(pfeat, d["f2w"], g, start=True, stop=True)
                    feat = sb.tile([128, R], bf16, tag=f"feat{i}", name=f"feat_{t}_{i}")
                    nc.vector.tensor_copy(out=feat, in_=pfeat)
                    fl.append(feat)
                    inp = [feat]
                feats[t] = fl

            # ---------- phase 4: level router softmaxes (all exps together)
            pls = {}
            for t in grp:
                inp = [hn[t][:, 0, :], hn[t][:, 1, :]]
                pl_t = []
                for i in range(3):
                    d = lv[i]
                    dk = LV_DK[i]
                    plog = psm.tile([4, R], f32, tag="small", name=f"plog_{t}_{i}")
                    for ko in range(dk):
                        nc.tensor.matmul(plog, d["wcomb"][:, ko, :], inp[ko],
                                         start=(ko == 0), stop=False)
                    nc.tensor.matmul(plog, d["rwf"], feats[t][i], start=False, stop=True)
                    e16 = sbs.tile([4, R], bf16, tag="e16", name=f"e16_{t}_{i}")
                    nc.scalar.activation(e16, plog, AF.Exp, bias=d["rb"])
                    pse = psm.tile([1, R], f32, tag="small", name=f"pse_{t}_{i}")
                    nc.tensor.matmul(pse, ones4, e16, start=True, stop=True)
                    rec = sbs.tile([1, R], f32, tag="rec", name=f"rec_{t}_{i}")
                    nc.vector.reciprocal_approx_fast(out=rec, in_=pse)
                    recb = sbs.tile([4, R], f32, tag="recb", name=f"recb_{t}_{i}")
                    nc.gpsimd.partition_broadcast(recb, rec, channels=4)
                    p_l = sb.tile([4, R], bf16, tag=f"p{i}", name=f"p_{t}_{i}")
                    nc.vector.tensor_tensor(p_l, e16, recb, OP.mult)
                    pl_t.append(p_l)
                    inp = [feats[t][i]]
                pls[t] = pl_t

            # ---------- phase 5: expert head + final softmax/argmax
            for t in grp:
                r0 = t * R
                ge = sb.tile([128, 2, R], bf16, tag="ge", name=f"ge_{t}")
                for mo in range(2):
                    pe1 = pbig.tile([128, R], f32, tag="big", name=f"pe1_{t}_{mo}")
                    nc.tensor.matmul(pe1, ehw1f[:, mo, :], feats[t][2], start=True, stop=False)
                    for l in range(3):
                        nc.tensor.matmul(pe1, ehw1p[l][:, mo, :], pls[t][l],
                                         start=False, stop=(l == 2))
                    nc.scalar.activation(ge[:, mo, :], pe1, AF.Gelu, bias=ehb1[:, mo:mo + 1])
                pl2 = pmx.tile([N_EXP, R], f32, tag="pmix", name=f"pl2_{t}")
                for mo in range(2):
                    nc.tensor.matmul(pl2, ehw2[:, mo, :], ge[:, mo, :],
                                     start=(mo == 0), stop=(mo == 1))
                lsb = sb.tile([N_EXP, R], f32, tag="lsb", name=f"lsb_{t}")
                nc.vector.tensor_scalar_add(lsb, pl2, ehb2)
                for sub in range(4):
                    pt = pmx.tile([128, N_EXP], f32, tag="pmix", name=f"pt_{t}_{sub}")
                    nc.tensor.transpose(pt, lsb[:, sub * 128:(sub + 1) * 128],
                                        ident[0:N_EXP, 0:N_EXP])
                    ls = sbs.tile([128, N_EXP], f32, tag="ls", name=f"ls_{t}_{sub}")
                    nc.vector.tensor_copy(out=ls, in_=pt)
                    m8 = sbs.tile([128, 8], f32, tag="m8", name=f"m8_{t}_{sub}")
                    nc.vector.max(m8, ls)
                    i8 = sbs.tile([128, 8], mybir.dt.uint32, tag="i8", name=f"i8_{t}_{sub}")
                    nc.vector.max_index(i8, m8, ls)
                    es = sbs.tile([128, N_EXP], f32, tag="es", name=f"es_{t}_{sub}")
                    ssum = sbs.tile([128, 1], f32, tag="ssum", name=f"ssum_{t}_{sub}")
                    nc.scalar.activation(es, ls, AF.Exp, accum_out=ssum)
                    srec = sbs.tile([128, 1], f32, tag="srec", name=f"srec_{t}_{sub}")
                    nc.vector.reciprocal_approx_fast(out=srec, in_=ssum)
                    prb = sbs.tile([128, N_EXP], f32, tag="prb", name=f"prb_{t}_{sub}")
                    nc.vector.tensor_scalar_mul(prb, es, srec)
                    nc.sync.dma_start(out=probs_d.ap()[r0 + sub * 128: r0 + (sub + 1) * 128, :],
                                      in_=prb)
                    c = t * 4 + sub
                    nc.vector.tensor_copy(out=idsf[:, c:c + 1], in_=i8[:, 0:1])

        pti = pmx.tile([NT, 128], f32, tag="pmix")
        nc.tensor.transpose(pti, idsf, ident)
        idsi = sbs.tile([NT, 128], mybir.dt.int32, tag="idsi")
        nc.vector.tensor_copy(out=idsi, in_=pti)
        nc.sync.dma_start(out=ids_d.ap(), in_=idsi)

    nc.finalize()
    return nc


def get_program():
    global _PROGRAM
    if _PROGRAM is None:
        _PROGRAM = _build_program()
    return _PROGRAM


def prep_inputs(inputs):
    """Host-side: fold geo term + all intermediate biases, cast to bf16, lay
    out weights for the SBUF layouts above; shard x. Returns 8 in_maps."""
    f32 = np.float32
    f64 = np.float64
    inp = {k: np.asarray(v) for k, v in inputs.items()}

    def c_(a):
        return np.ascontiguousarray(a)

    shared = {}
    shared["w1"] = inp["ipW1"].reshape(KO, 128, 4, 128).transpose(1, 0, 2, 3).astype(BF16)
    shared["b1"] = c_(inp["ipb1"].astype(f32).reshape(4, 128).T)
    shared["w2"] = inp["ipW2"].reshape(4, 128, 2, 128).transpose(1, 0, 2, 3).astype(BF16)
    shared["b2"] = c_(inp["ipb2"].astype(f32).reshape(2, 128).T)
    shared["lng"] = c_(inp["ln_g"].astype(f32).reshape(2, 128).T)
    shared["lnb"] = c_(inp["ln_b"].astype(f32).reshape(2, 128).T)

    # bias of the previous level's feat output, folded into this level's mats
    prev_f2b = None  # f2b of level i-1 (None for level 1: LN bias is on-chip)
    for i in range(3):
        p = f"l{i + 1}"
        dk = LV_DK[i]
        W3d = inp[p + "W3d"].astype(f64)          # [din, 3]
        b3d = inp[p + "b3d"].astype(f64)          # [3]
        cc = inp[p + "c"].astype(f64)             # [4, 3]
        rW = inp[p + "rW"].astype(f64)            # [FD+3, 4]
        rb = inp[p + "rb"].astype(f64)            # [4]
        f1W = inp[p + "f1W"].astype(f64)          # [din, FD]
        f1b = inp[p + "f1b"].astype(f64)          # [FD]
        f2b = inp[p + "f2b"].astype(f64)          # [FD]

        rwp = rW[FD:FD + 3] + 2.0 * GS * cc.T     # [3, 4]
        wcomb = W3d @ rwp                         # [din, 4]
        rb2 = rb - GS * (cc ** 2).sum(1) + b3d @ rwp + rW[:FD].T @ f2b
        f1b2 = f1b.copy()
        if prev_f2b is not None:
            rb2 = rb2 + wcomb.T @ prev_f2b
            f1b2 = f1b2 + f1W.T @ prev_f2b
        shared[f"l{i}wcomb"] = wcomb.astype(f32).reshape(dk, 128, 4).transpose(1, 0, 2).astype(BF16)
        shared[f"l{i}rwf"] = inp[p + "rW"][:FD].astype(BF16)
        shared[f"l{i}rb"] = c_(rb2.astype(f32).reshape(4, 1))
        shared[f"l{i}f1w"] = inp[p + "f1W"].reshape(dk, 128, 128).transpose(1, 0, 2).astype(BF16)
        shared[f"l{i}f1b"] = c_(f1b2.astype(f32).reshape(128, 1))
        shared[f"l{i}f2w"] = inp[p + "f2W"].astype(BF16)
        prev_f2b = f2b

    ehW1 = inp["ehW1"].astype(f64)
    ehb1 = inp["ehb1"].astype(f64) + ehW1[:FD].T @ prev_f2b
    shared["ehw1f"] = inp["ehW1"][:FD].reshape(128, 2, 128).astype(BF16)
    for l in range(3):
        shared[f"ehw1p{l}"] = inp["ehW1"][FD + 4 * l:FD + 4 * l + 4].reshape(4, 2, 128).astype(BF16)
    shared["ehb1"] = c_(ehb1.astype(f32).reshape(2, 128).T)
    shared["ehw2"] = inp["ehW2"].reshape(2, 128, N_EXP).transpose(1, 0, 2).astype(BF16)
    shared["ehb2"] = c_(inp["ehb2"].astype(f32).reshape(N_EXP, 1))

    xb = inp["x"].astype(BF16)
    maps = []
    for core in range(N_CORES):
        xc = xb[core * BLOC:(core + 1) * BLOC]            # [BLOC, D_IN]
        # [t, p, ko, r] = x[t*R + r, ko*128 + p]  (contiguous 16KB/partition)
        xT = c_(xc.reshape(NRT, R, KO, 128).transpose(0, 3, 2, 1))
        m = dict(shared)
        m["xT"] = xT
        maps.append(m)
    return maps


def run(inputs, trace=False, **kw):
    from concourse.bass_utils import run_bass_kernel_spmd
    nc = get_program()
    in_maps = prep_inputs(inputs)
    res = run_bass_kernel_spmd(nc, in_maps, core_ids=list(range(N_CORES)),
                               trace=trace, **kw)
    probs = np.concatenate([r["probs"] for r in res.results], axis=0)
    ids = np.concatenate([np.ascontiguousarray(r["ids"]).reshape(-1)
                          for r in res.results], axis=0).astype(np.int32)
    return (probs, ids), res


def kernel(**inputs):
    (probs, ids), _ = run(inputs, trace=False)
    return probs, ids


# revision 10
# speedup vs baseline: 1.5736x; 1.1299x over previous
"""Trainium2 Bass kernel for nn_EnhancedBVHRouter (hierarchical MoE router).

Strategy:
  - Pure data parallel over batch: 32768 rows -> 8 cores x 4096 rows.
  - All activations kept TRANSPOSED on-chip ([features on partitions, rows on
    free dim]) so every linear layer is `out_T = W^T @ in_T` with the weight
    matrix as the stationary matmul operand; no activation transposes in the
    main chain.  Matmul operands are bf16 (full PE rate), f32 PSUM accumulate.
  - The level geometric term and every intermediate bias are folded on the
    host:  0.5*geo_j = -gs*|pos|^2 + 2*gs*pos.c_j - gs*|c_j|^2 drops its
    row-constant |pos|^2 term (softmax invariant); the pos path itself is
    folded into one combined matrix Wcomb = W3d @ rW_pos', so `pos` is never
    materialized on chip.
  - LayerNorm / level-softmax partition reductions are ones-matmuls on the PE
    (ones scaled 1/256 so the PSUM result IS the mean); per-row scalars come
    back via gpsimd.partition_broadcast; reciprocals use the single-op
    reciprocal_approx_fast (~5x faster than InstReciprocal).
  - Two row-tiles are processed per "group" phase-by-phase so same-function
    ACT ops batch together (the Gelu/Exp/Sqrt LUT table reload costs ~1.3us
    per switch).
  - Final 64-way softmax (no max subtraction needed: |logits| < 0.01) +
    argmax run row-major after a PE transpose of the [64, R] logits.
"""

import os
import sys

import numpy as np

for _p in ("/opt/trn_rl_repo", "/root/.axon_site/_ro/trn_rl_repo"):
    if os.path.isdir(_p) and _p not in sys.path:
        sys.path.append(_p)

import ml_dtypes

BF16 = ml_dtypes.bfloat16

B, D_IN, FD = 32768, 2048, 128
N_CORES = 8
BLOC = B // N_CORES          # 4096 rows per core
R = 512                      # rows per macro tile
NRT = BLOC // R              # 8 macro tiles per core
KO = D_IN // 128             # 16 k-chunks for the input projection
NT = BLOC // 128             # 32 argmax columns per core
GRP = 2                      # row-tiles per phase group
T_BUF = 1.0
LN_EPS = 1e-5
GS = 0.5 / (2.0 * T_BUF * T_BUF + 1e-8)
N_EXP = 64
LV_DK = (2, 1, 1)            # k-chunks of each level's input

_PROGRAM = None


def _build_program():
    import concourse.tile as tile
    import concourse.mybir as mybir
    from concourse import bacc
    from concourse.masks import make_identity
    from contextlib import ExitStack

    f32 = mybir.dt.float32
    bf16 = mybir.dt.bfloat16
    AF = mybir.ActivationFunctionType
    OP = mybir.AluOpType

    nc = bacc.Bacc("TRN2", target_bir_lowering=False, debug=False)

    # ---- DRAM tensors (names are the in_map keys) ----
    xT_d = nc.dram_tensor("xT", [NRT, 128, KO, R], bf16, kind="ExternalInput")
    w1_d = nc.dram_tensor("w1", [128, KO, 4, 128], bf16, kind="ExternalInput")
    b1_d = nc.dram_tensor("b1", [128, 4], f32, kind="ExternalInput")
    w2_d = nc.dram_tensor("w2", [128, 4, 2, 128], bf16, kind="ExternalInput")
    b2_d = nc.dram_tensor("b2", [128, 2], f32, kind="ExternalInput")
    lng_d = nc.dram_tensor("lng", [128, 2], f32, kind="ExternalInput")
    lnb_d = nc.dram_tensor("lnb", [128, 2], f32, kind="ExternalInput")
    lv_d = []
    for i in range(3):
        dk = LV_DK[i]
        lv_d.append({
            "f1w": nc.dram_tensor(f"l{i}f1w", [128, dk, 128], bf16, kind="ExternalInput"),
            "f1b": nc.dram_tensor(f"l{i}f1b", [128, 1], f32, kind="ExternalInput"),
            "f2w": nc.dram_tensor(f"l{i}f2w", [128, 128], bf16, kind="ExternalInput"),
            "wcomb": nc.dram_tensor(f"l{i}wcomb", [128, dk, 4], bf16, kind="ExternalInput"),
            "rwf": nc.dram_tensor(f"l{i}rwf", [128, 4], bf16, kind="ExternalInput"),
            "rb": nc.dram_tensor(f"l{i}rb", [4, 1], f32, kind="ExternalInput"),
        })
    ehw1f_d = nc.dram_tensor("ehw1f", [128, 2, 128], bf16, kind="ExternalInput")
    ehw1p_d = [nc.dram_tensor(f"ehw1p{l}", [4, 2, 128], bf16, kind="ExternalInput")
               for l in range(3)]
    ehb1_d = nc.dram_tensor("ehb1", [128, 2], f32, kind="ExternalInput")
    ehw2_d = nc.dram_tensor("ehw2", [128, 2, N_EXP], bf16, kind="ExternalInput")
    ehb2_d = nc.dram_tensor("ehb2", [N_EXP, 1], f32, kind="ExternalInput")

    probs_d = nc.dram_tensor("probs", [BLOC, N_EXP], f32, kind="ExternalOutput")
    ids_d = nc.dram_tensor("ids", [NT, 128], mybir.dt.int32, kind="ExternalOutput")

    with tile.TileContext(nc) as tc, ExitStack() as ctx:
        from concourse.bass import _add_dep_helper

        singles = ctx.enter_context(tc.tile_pool(name="singles", bufs=1))
        sb = ctx.enter_context(tc.tile_pool(name="sb", bufs=2))
        sbs = ctx.enter_context(tc.tile_pool(name="sbs", bufs=3))
        pbig = ctx.enter_context(tc.tile_pool(name="pbig", bufs=4, space="PSUM"))
        psm = ctx.enter_context(tc.tile_pool(name="psm", bufs=2, space="PSUM"))
        pmx = ctx.enter_context(tc.tile_pool(name="pmx", bufs=2, space="PSUM"))

        acts = []

        def act(*a, **kw):
            """scalar.activation with a same-engine ordering chain so the
            scheduler can't interleave LUT-table families."""
            inst = nc.scalar.activation(*a, **kw)
            if acts:
                _add_dep_helper(inst.ins, acts[-1].ins, sync=False,
                                reason="act table order")
            acts.append(inst)
            return inst

        def load(dram, shape, dtype):
            t = singles.tile(shape, dtype, tag=f"w_{dram.name}")
            nc.sync.dma_start(out=t, in_=dram.ap())
            return t

        # prefetch the first group of x tiles before the weight loads
        xt = {}

        def fetch_xt(t):
            xt[t] = sb.tile([128, KO, R], bf16, tag="xt", bufs=4, name=f"xt{t}")
            nc.sync.dma_start(out=xt[t], in_=xT_d.ap()[t])

        for t in range(min(GRP, NRT)):
            fetch_xt(t)

        w1 = load(w1_d, [128, KO, 4, 128], bf16)
        b1 = load(b1_d, [128, 4], f32)
        w2 = load(w2_d, [128, 4, 2, 128], bf16)
        b2 = load(b2_d, [128, 2], f32)
        lng = load(lng_d, [128, 2], f32)
        lnb = load(lnb_d, [128, 2], f32)
        lv = []
        for i, d in enumerate(lv_d):
            dk = LV_DK[i]
            lv.append({
                "f1w": load(d["f1w"], [128, dk, 128], bf16),
                "f1b": load(d["f1b"], [128, 1], f32),
                "f2w": load(d["f2w"], [128, 128], bf16),
                "wcomb": load(d["wcomb"], [128, dk, 4], bf16),
                "rwf": load(d["rwf"], [128, 4], bf16),
                "rb": load(d["rb"], [4, 1], f32),
            })
        ehw1f = load(ehw1f_d, [128, 2, 128], bf16)
        ehw1p = [load(d, [4, 2, 128], bf16) for d in ehw1p_d]
        ehb1 = load(ehb1_d, [128, 2], f32)
        ehw2 = load(ehw2_d, [128, 2, N_EXP], bf16)
        ehb2 = load(ehb2_d, [N_EXP, 1], f32)

        # ones/256 so the stats matmuls produce means directly
        onesm = singles.tile([128, 1], bf16)
        nc.vector.memset(onesm, 1.0 / 256.0)
        ones4 = singles.tile([4, 1], bf16)
        nc.vector.memset(ones4, 1.0)
        epsb = singles.tile([1, 1], f32)
        nc.vector.memset(epsb, LN_EPS)
        ident = singles.tile([128, 128], f32)
        make_identity(nc, ident)
        idsf = singles.tile([128, NT], f32)

        for tp in range(0, NRT, GRP):
            grp = list(range(tp, min(tp + GRP, NRT)))
            for t in range(tp + GRP, min(tp + 2 * GRP, NRT)):
                fetch_xt(t)  # prefetch next group

            # ---------- phase 1: input_proj layer 1 (t innermost: weight reuse)
            g1 = {t: sb.tile([128, 4, R], bf16, tag="g1", bufs=2, name=f"g1_{t}")
                  for t in grp}
            for mo in range(4):
                ph = {t: pbig.tile([128, R], f32, tag="big", name=f"ph1_{t}_{mo}")
                      for t in grp}
                for ko in range(KO):
                    for t in grp:
                        nc.tensor.matmul(ph[t], w1[:, ko, mo, :], xt[t][:, ko, :],
                                         start=(ko == 0), stop=(ko == KO - 1))
                for t in grp:
                    act(g1[t][:, mo, :], ph[t], AF.Gelu, bias=b1[:, mo:mo + 1])

            # ---------- phase 2: input_proj layer 2 + LayerNorm
            h2 = {}
            hn = {}
            for t in grp:
                h2[t] = sb.tile([128, 2, R], bf16, tag="h2", name=f"h2_{t}")
            for mo in range(2):
                ph = {t: pbig.tile([128, R], f32, tag="big", name=f"ph2_{t}_{mo}")
                      for t in grp}
                for ko in range(4):
                    for t in grp:
                        nc.tensor.matmul(ph[t], w2[:, ko, mo, :], g1[t][:, ko, :],
                                         start=(ko == 0), stop=(ko == 3))
                for t in grp:
                    nc.vector.tensor_scalar_add(h2[t][:, mo, :], ph[t], b2[:, mo:mo + 1])
            for t in grp:
                h2q = sb.tile([128, 2, R], bf16, tag="h2q", name=f"h2q_{t}")
                for mo in range(2):
                    nc.vector.tensor_tensor(h2q[:, mo, :], h2[t][:, mo, :], h2[t][:, mo, :], OP.mult)
                mu_ps = psm.tile([1, R], f32, tag="small", name=f"mu_ps_{t}")
                for mo in range(2):
                    nc.tensor.matmul(mu_ps, onesm, h2[t][:, mo, :], start=(mo == 0), stop=(mo == 1))
                e2_ps = psm.tile([1, R], f32, tag="small", name=f"e2_ps_{t}")
                for mo in range(2):
                    nc.tensor.matmul(e2_ps, onesm, h2q[:, mo, :], start=(mo == 0), stop=(mo == 1))
                musb = sbs.tile([1, R], f32, tag="musb", name=f"musb_{t}")
                nc.vector.tensor_copy(out=musb, in_=mu_ps)
                mu16 = sbs.tile([1, R], bf16, tag="mu16", name=f"mu16_{t}")
                nc.vector.tensor_copy(out=mu16, in_=musb)
                var = sbs.tile([1, R], f32, tag="var", name=f"var_{t}")
                nc.vector.tensor_tensor(var, musb, musb, OP.mult)
                nc.vector.tensor_tensor(var, e2_ps, var, OP.subtract)
                sd = sbs.tile([1, R], f32, tag="sd", name=f"sd_{t}")
                act(sd, var, AF.Sqrt, bias=epsb)
                rstd = sbs.tile([1, R], f32, tag="rstd", name=f"rstd_{t}")
                nc.vector.reciprocal_approx_fast(out=rstd, in_=sd)
                rstd16 = sbs.tile([1, R], bf16, tag="rstd16", name=f"rstd16_{t}")
                nc.vector.tensor_copy(out=rstd16, in_=rstd)
                mub = sb.tile([128, R], bf16, tag="mub", name=f"mub_{t}")
                nc.gpsimd.partition_broadcast(mub, mu16, channels=128)
                rstdb = sb.tile([128, R], bf16, tag="rstdb", name=f"rstdb_{t}")
                nc.gpsimd.partition_broadcast(rstdb, rstd16, channels=128)
                hn[t] = sb.tile([128, 2, R], bf16, tag="hn", name=f"hn_{t}")
                for mo in range(2):
                    zc = sb.tile([128, R], bf16, tag="zc", name=f"zc_{t}_{mo}")
                    nc.vector.tensor_tensor(zc, h2[t][:, mo, :], mub, OP.subtract)
                    nc.vector.tensor_tensor(zc, zc, rstdb, OP.mult)
                    nc.vector.tensor_scalar(out=hn[t][:, mo, :], in0=zc,
                                            scalar1=lng[:, mo:mo + 1], scalar2=lnb[:, mo:mo + 1],
                                            op0=OP.mult, op1=OP.add)

            # ---------- phase 3: level feature chains (all gelus together)
            feats = {t: [] for t in grp}
            inp = {t: [hn[t][:, 0, :], hn[t][:, 1, :]] for t in grp}
            for i in range(3):
                d = lv[i]
                dk = LV_DK[i]
                pf = {}
                for t in grp:
                    pf[t] = pbig.tile([128, R], f32, tag="big", name=f"pf_{t}_{i}")
                    for ko in range(dk):
                        nc.tensor.matmul(pf[t], d["f1w"][:, ko, :], inp[t][ko],
                                         start=(ko == 0), stop=(ko == dk - 1))
                g = {}
                for t in grp:
                    g[t] = sbs.tile([128, R], bf16, tag="g", name=f"g_{t}_{i}")
                    act(g[t], pf[t], AF.Gelu, bias=d["f1b"])
                for t in grp:
                    pfeat = pbig.tile([128, R], f32, tag="big", name=f"pfeat_{t}_{i}")
                    nc.tensor.matmul(pfeat, d["f2w"], g[t], start=True, stop=True)
                    feat = sb.tile([128, R], bf16, tag=f"feat{i}", name=f"feat_{t}_{i}")
                    nc.vector.tensor_copy(out=feat, in_=pfeat)
                    feats[t].append(feat)
                    inp[t] = [feat]

            # ---------- phase 4: level router softmaxes (all exps together)
            pls = {t: [] for t in grp}
            inp = {t: [hn[t][:, 0, :], hn[t][:, 1, :]] for t in grp}
            plogs = {}
            for i in range(3):
                d = lv[i]
                dk = LV_DK[i]
                for t in grp:
                    plog = psm.tile([4, R], f32, tag="small", name=f"plog_{t}_{i}")
                    for ko in range(dk):
                        nc.tensor.matmul(plog, d["wcomb"][:, ko, :], inp[t][ko],
                                         start=(ko == 0), stop=False)
                    nc.tensor.matmul(plog, d["rwf"], feats[t][i], start=False, stop=True)
                    plogs[t] = plog
                for t in grp:
                    e16 = sbs.tile([4, R], bf16, tag="e16", name=f"e16_{t}_{i}")
                    act(e16, plogs[t], AF.Exp, bias=d["rb"])
                    pse = psm.tile([1, R], f32, tag="small", name=f"pse_{t}_{i}")
                    nc.tensor.matmul(pse, ones4, e16, start=True, stop=True)
                    rec = sbs.tile([1, R], f32, tag="rec", name=f"rec_{t}_{i}")
                    nc.vector.reciprocal_approx_fast(out=rec, in_=pse)
                    recb = sbs.tile([4, R], f32, tag="recb", name=f"recb_{t}_{i}")
                    nc.gpsimd.partition_broadcast(recb, rec, channels=4)
                    p_l = sb.tile([4, R], bf16, tag=f"p{i}", name=f"p_{t}_{i}")
                    nc.vector.tensor_tensor(p_l, e16, recb, OP.mult)
                    pls[t].append(p_l)
                    inp[t] = [feats[t][i]]

            # ---------- phase 5a: expert head matmuls + gelus
            ge = {}
            for mo in range(2):
                pe1 = {}
                for t in grp:
                    pe1[t] = pbig.tile([128, R], f32, tag="big", name=f"pe1_{t}_{mo}")
                    nc.tensor.matmul(pe1[t], ehw1f[:, mo, :], feats[t][2], start=True, stop=False)
                    for l in range(3):
                        nc.tensor.matmul(pe1[t], ehw1p[l][:, mo, :], pls[t][l],
                                         start=False, stop=(l == 2))
                for t in grp:
                    if t not in ge:
                        ge[t] = sb.tile([128, 2, R], bf16, tag="ge", name=f"ge_{t}")
                    act(ge[t][:, mo, :], pe1[t], AF.Gelu, bias=ehb1[:, mo:mo + 1])

            # ---------- phase 5b: final logits, softmax, argmax
            for t in grp:
                r0 = t * R
                pl2 = pmx.tile([N_EXP, R], f32, tag="pmix", name=f"pl2_{t}")
                for mo in range(2):
                    nc.tensor.matmul(pl2, ehw2[:, mo, :], ge[t][:, mo, :],
                                     start=(mo == 0), stop=(mo == 1))
                lsb = sb.tile([N_EXP, R], f32, tag="lsb", name=f"lsb_{t}")
                nc.vector.tensor_scalar_add(lsb, pl2, ehb2)
                for sub in range(4):
                    pt = pmx.tile([128, N_EXP], f32, tag="pmix", name=f"pt_{t}_{sub}")
                    nc.tensor.transpose(pt, lsb[:, sub * 128:(sub + 1) * 128],
                                        ident[0:N_EXP, 0:N_EXP])
                    ls = sbs.tile([128, N_EXP], f32, tag="ls", name=f"ls_{t}_{sub}")
                    nc.vector.tensor_copy(out=ls, in_=pt)
                    m8 = sbs.tile([128, 8], f32, tag="m8", name=f"m8_{t}_{sub}")
                    nc.vector.max(m8, ls)
                    i8 = sbs.tile([128, 8], mybir.dt.uint32, tag="i8", name=f"i8_{t}_{sub}")
                    nc.vector.max_index(i8, m8, ls)
                    es = sbs.tile([128, N_EXP], f32, tag="es", name=f"es_{t}_{sub}")
                    ssum = sbs.tile([128, 1], f32, tag="ssum", name=f"ssum_{t}_{sub}")
                    act(es, ls, AF.Exp, accum_out=ssum)
                    srec = sbs.tile([128, 1], f32, tag="srec", name=f"srec_{t}_{sub}")
                    nc.vector.reciprocal_approx_fast(out=srec, in_=ssum)
                    prb = sbs.tile([128, N_EXP], f32, tag="prb", name=f"prb_{t}_{sub}")
                    nc.vector.tensor_scalar_mul(prb, es, srec)
                    nc.sync.dma_start(out=probs_d.ap()[r0 + sub * 128: r0 + (sub + 1) * 128, :],
                                      in_=prb)
                    c = t * 4 + sub
                    nc.vector.tensor_copy(out=idsf[:, c:c + 1], in_=i8[:, 0:1])

        pti = pmx.tile([NT, 128], f32, tag="pmix")
        nc.tensor.transpose(pti, idsf, ident)
        idsi = sbs.tile([NT, 128], mybir.dt.int32, tag="idsi")
        nc.vector.tensor_copy(out=idsi, in_=pti)
        nc.sync.dma_start(out=ids_d.ap(), in_=idsi)

    nc.finalize()
    return nc


def get_program():
    global _PROGRAM
    if _PROGRAM is None:
        _PROGRAM = _build_program()
    return _PROGRAM


def prep_inputs(inputs):
    """Host-side: fold geo term + all intermediate biases, cast to bf16, lay
    out weights for the SBUF layouts above; shard x. Returns 8 in_maps."""
    f32 = np.float32
    f64 = np.float64
    inp = {k: np.asarray(v) for k, v in inputs.items()}

    def c_(a):
        return np.ascontiguousarray(a)

    shared = {}
    shared["w1"] = inp["ipW1"].reshape(KO, 128, 4, 128).transpose(1, 0, 2, 3).astype(BF16)
    shared["b1"] = c_(inp["ipb1"].astype(f32).reshape(4, 128).T)
    shared["w2"] = inp["ipW2"].reshape(4, 128, 2, 128).transpose(1, 0, 2, 3).astype(BF16)
    shared["b2"] = c_(inp["ipb2"].astype(f32).reshape(2, 128).T)
    shared["lng"] = c_(inp["ln_g"].astype(f32).reshape(2, 128).T)
    shared["lnb"] = c_(inp["ln_b"].astype(f32).reshape(2, 128).T)

    # bias of the previous level's feat output, folded into this level's mats
    prev_f2b = None  # f2b of level i-1 (None for level 1: LN bias is on-chip)
    for i in range(3):
        p = f"l{i + 1}"
        dk = LV_DK[i]
        W3d = inp[p + "W3d"].astype(f64)          # [din, 3]
        b3d = inp[p + "b3d"].astype(f64)          # [3]
        cc = inp[p + "c"].astype(f64)             # [4, 3]
        rW = inp[p + "rW"].astype(f64)            # [FD+3, 4]
        rb = inp[p + "rb"].astype(f64)            # [4]
        f1W = inp[p + "f1W"].astype(f64)          # [din, FD]
        f1b = inp[p + "f1b"].astype(f64)          # [FD]
        f2b = inp[p + "f2b"].astype(f64)          # [FD]

        rwp = rW[FD:FD + 3] + 2.0 * GS * cc.T     # [3, 4]
        wcomb = W3d @ rwp                         # [din, 4]
        rb2 = rb - GS * (cc ** 2).sum(1) + b3d @ rwp + rW[:FD].T @ f2b
        f1b2 = f1b.copy()
        if prev_f2b is not None:
            rb2 = rb2 + wcomb.T @ prev_f2b
            f1b2 = f1b2 + f1W.T @ prev_f2b
        shared[f"l{i}wcomb"] = wcomb.astype(f32).reshape(dk, 128, 4).transpose(1, 0, 2).astype(BF16)
        shared[f"l{i}rwf"] = inp[p + "rW"][:FD].astype(BF16)
        shared[f"l{i}rb"] = c_(rb2.astype(f32).reshape(4, 1))
        shared[f"l{i}f1w"] = inp[p + "f1W"].reshape(dk, 128, 128).transpose(1, 0, 2).astype(BF16)
        shared[f"l{i}f1b"] = c_(f1b2.astype(f32).reshape(128, 1))
        shared[f"l{i}f2w"] = inp[p + "f2W"].astype(BF16)
        prev_f2b = f2b

    ehW1 = inp["ehW1"].astype(f64)
    ehb1 = inp["ehb1"].astype(f64) + ehW1[:FD].T @ prev_f2b
    shared["ehw1f"] = inp["ehW1"][:FD].reshape(128, 2, 128).astype(BF16)
    for l in range(3):
        shared[f"ehw1p{l}"] = inp["ehW1"][FD + 4 * l:FD + 4 * l + 4].reshape(4, 2, 128).astype(BF16)
    shared["ehb1"] = c_(ehb1.astype(f32).reshape(2, 128).T)
    shared["ehw2"] = inp["ehW2"].reshape(2, 128, N_EXP).transpose(1, 0, 2).astype(BF16)
    shared["ehb2"] = c_(inp["ehb2"].astype(f32).reshape(N_EXP, 1))

    xb = inp["x"].astype(BF16)
    maps = []
    for core in range(N_CORES):
        xc = xb[core * BLOC:(core + 1) * BLOC]            # [BLOC, D_IN]
        # [t, p, ko, r] = x[t*R + r, ko*128 + p]  (contiguous 16KB/partition)
        xT = c_(xc.reshape(NRT, R, KO, 128).transpose(0, 3, 2, 1))
        m = dict(shared)
        m["xT"] = xT
        maps.append(m)
    return maps


def run(inputs, trace=False, **kw):
    from concourse.bass_utils import run_bass_kernel_spmd
    nc = get_program()
    in_maps = prep_inputs(inputs)
    res = run_bass_kernel_spmd(nc, in_maps, core_ids=list(range(N_CORES)),
                               trace=trace, **kw)
    probs = np.concatenate([r["probs"] for r in res.results], axis=0)
    ids = np.concatenate([np.ascontiguousarray(r["ids"]).reshape(-1)
                          for r in res.results], axis=0).astype(np.int32)
    return (probs, ids), res


def kernel(**inputs):
    (probs, ids), _ = run(inputs, trace=False)
    return probs, ids
